# revision 2
# baseline (speedup 1.0000x reference)
"""CheckInEmbedding kernel for Trainium2 (8 NeuronCores, data-parallel).

reference:
    poi = leaky_relu(cat([hotness, region], axis=1), slope=0.2)   # [N, 128]
    out = cat([poi, broadcast(user, (N, 128))], axis=1)           # [N, 256]

Strategy (memory-bound, ~96 MB HBM traffic per core):
  * Host: concat hotness+region -> poi [N, 128]; shard N across 8 cores
    (62500 rows each, padded to 62592 = 489*128).
  * Device, per tile of R rows/partition:
      load  poi -> A        (contiguous on BOTH sides: 24 KB/partition runs)
      DVE   leaky_relu A -> B's poi slots (strided SBUF write; DVE handles
            the interleave so no DMA ever touches sub-KB runs)
      store B -> out        (contiguous 48 KB/partition runs)
    B's user-embedding slots are pre-filled once per buffer; stores only
    read them, so they stay valid across buffer reuse.
  * vs. the previous version (DMA straight into the interleaved layout):
    the load's SBUF-side runs were 512 B -> 6144 descriptors per tile,
    ~67k per pass; descriptor generation/handling dominated. Now each DMA
    is 128 descriptors of 24-48 KB.
"""

import numpy as np

N = 500000
DPOI = 128  # hotness(64) + region(64)
DU = 128
DOUT = DPOI + DU
NCORES = 8
ROWS_PER_CORE = N // NCORES  # 62500
GROUPS = 489  # ceil(62500 / 128)
PAD_ROWS = GROUPS * 128  # 62592
TILE_SCHEDULE = [48] * 10 + [9]
NBUFS = 2  # A/B pairs; 2*(24K + 48K) = 144 KB/partition, fits in 208 KB

_prog_cache = {}


def _emit_pass(nc, mybir, abufs, bbufs, poi, out, tile_schedule):
    nbufs = len(abufs)
    row0 = 0
    for i, r in enumerate(tile_schedule):
        a = abufs[i % nbufs]
        b = bbufs[i % nbufs]
        rows = r * 128
        # load: HBM contiguous (24 KB/partition) -> SBUF contiguous
        src = poi[row0 : row0 + rows, :].rearrange("(p q) d -> p (q d)", q=r)
        nc.sync.dma_start(out=a[:, 0 : r * DPOI], in_=src)
        # leaky_relu(x) = max(0.2*x, x), interleaving into B's poi slots
        av = a[:, 0 : r * DPOI].rearrange("p (q d) -> p q d", q=r)
        bv = b[:].rearrange("p (q c) -> p q c", c=DOUT)
        nc.vector.scalar_tensor_tensor(
            out=bv[:, 0:r, 0:DPOI],
            in0=av,
            scalar=0.2,
            in1=av,
            op0=mybir.AluOpType.mult,
            op1=mybir.AluOpType.max,
        )
        # store: SBUF contiguous -> HBM contiguous (48 KB/partition)
        dst = out[row0 : row0 + rows, :].rearrange("(p q) c -> p (q c)", q=r)
        nc.scalar.dma_start(out=dst, in_=b[:, 0 : r * DOUT])
        row0 += rows


def _build_program(pad_rows, tile_schedule, nbufs, repeats=1):
    import concourse.bacc as bacc
    import concourse.mybir as mybir
    from concourse.tile import TileContext

    f32 = mybir.dt.float32
    nc = bacc.Bacc()
    poi = nc.declare_dram_parameter("poi", [pad_rows, DPOI], f32, isOutput=False)
    ublk = nc.declare_dram_parameter("ublk", [128, DU], f32, isOutput=False)
    out = nc.declare_dram_parameter("out", [pad_rows, DOUT], f32, isOutput=True)

    rmax = max(tile_schedule)
    with TileContext(nc) as tc:
        with (
            tc.tile_pool(name="abuf", bufs=1) as apool,
            tc.tile_pool(name="bbuf", bufs=1) as bpool,
            tc.tile_pool(name="ubuf", bufs=1) as upool,
        ):
            usr = upool.tile([128, DU], f32)
            nc.sync.dma_start(out=usr[:], in_=ublk[:])

            abufs = [
                apool.tile([128, rmax * DPOI], f32, name=f"abuf{i}")
                for i in range(nbufs)
            ]
            bbufs = [
                bpool.tile([128, rmax * DOUT], f32, name=f"bbuf{i}")
                for i in range(nbufs)
            ]
            # Pre-fill the user-embedding slots of every B buffer once:
            # seed row-slot 0 from usr, then doubling copies, all on DVE so
            # each store's producers live on one engine.
            for b in bbufs:
                bv = b[:].rearrange("p (q c) -> p q c", c=DOUT)
                nc.vector.tensor_copy(
                    out=bv[:, 0:1, DPOI:DOUT],
                    in_=usr[:].rearrange("p (q c) -> p q c", q=1),
                )
                q = 1
                while q < rmax:
                    step = min(q, rmax - q)
                    nc.vector.tensor_copy(
                        out=bv[:, q : q + step, DPOI:DOUT],
                        in_=bv[:, 0:step, DPOI:DOUT],
                    )
                    q += step

            for _ in range(repeats):
                _emit_pass(nc, mybir, abufs, bbufs, poi, out, tile_schedule)
    nc.compile()
    return nc


def _get_program(pad_rows, tile_schedule, nbufs, repeats=1):
    key = (pad_rows, tuple(tile_schedule), nbufs, repeats)
    if key not in _prog_cache:
        _prog_cache[key] = _build_program(pad_rows, tile_schedule, nbufs, repeats)
    return _prog_cache[key]


def _prepare(hot, reg, user, rows_per_core, pad_rows, tile_schedule, nbufs, repeats=1):
    nc = _get_program(pad_rows, tile_schedule, nbufs, repeats)
    poi_full = np.concatenate(
        [np.ascontiguousarray(hot), np.ascontiguousarray(reg)], axis=1
    ).astype(np.float32, copy=False)
    ublk = np.broadcast_to(
        np.asarray(user, dtype=np.float32).reshape(1, DU), (128, DU)
    ).copy()
    in_maps = []
    for c in range(NCORES):
        sl = poi_full[c * rows_per_core : (c + 1) * rows_per_core]
        if pad_rows != rows_per_core:
            p = np.zeros((pad_rows, DPOI), np.float32)
            p[:rows_per_core] = sl
        else:
            p = np.ascontiguousarray(sl)
        in_maps.append({"poi": p, "ublk": ublk})
    return nc, in_maps


def _run(hot, reg, user, rows_per_core, pad_rows, tile_schedule, nbufs, **spmd_kwargs):
    from concourse.bass_utils import run_bass_kernel_spmd

    nc, in_maps = _prepare(
        hot, reg, user, rows_per_core, pad_rows, tile_schedule, nbufs
    )
    res = run_bass_kernel_spmd(nc, in_maps, list(range(NCORES)), **spmd_kwargs)
    outs = [res.results[c]["out"][:rows_per_core] for c in range(NCORES)]
    return np.concatenate(outs, axis=0), res


def kernel(hotness_embedding_list, region_embedding_list, user_embedding):
    out, _ = _run(
        hotness_embedding_list,
        region_embedding_list,
        user_embedding,
        ROWS_PER_CORE,
        PAD_ROWS,
        TILE_SCHEDULE,
        NBUFS,
    )
    return out


# revision 3
# speedup vs baseline: 1.3482x; 1.3482x over previous
"""CheckInEmbedding kernel for Trainium2 (8 NeuronCores, data-parallel).

reference:
    poi = leaky_relu(cat([hotness, region], axis=1), slope=0.2)   # [N, 128]
    out = cat([poi, broadcast(user, (N, 128))], axis=1)           # [N, 256]

Strategy (memory-bound, ~96 MB HBM traffic per core):
  * Host: concat hotness+region -> poi [N, 128]; shard N across 8 cores
    (62500 rows each, padded to 62592 = 489*128).
  * Device, per tile of R rows/partition:
      load  poi -> A        (contiguous on BOTH sides: 24 KB/partition runs)
      DVE   leaky_relu A -> B's poi slots (strided SBUF write; DVE handles
            the interleave so no DMA ever touches sub-KB runs)
      store B -> out        (contiguous 48 KB/partition runs)
    B's user-embedding slots are pre-filled once per buffer; stores only
    read them, so they stay valid across buffer reuse.
  * vs. the previous version (DMA straight into the interleaved layout):
    the load's SBUF-side runs were 512 B -> 6144 descriptors per tile,
    ~67k per pass; descriptor generation/handling dominated. Now each DMA
    is 128 descriptors of 24-48 KB.
"""

import numpy as np

N = 500000
DPOI = 128  # hotness(64) + region(64)
DU = 128
DOUT = DPOI + DU
NCORES = 8
ROWS_PER_CORE = N // NCORES  # 62500
GROUPS = 489  # ceil(62500 / 128)
PAD_ROWS = GROUPS * 128  # 62592
TILE_SCHEDULE = [48] * 10 + [9]
NBUFS = 2  # A/B pairs; 2*(24K + 48K) = 144 KB/partition, fits in 208 KB

_prog_cache = {}


def _emit_pass(nc, mybir, abufs, bbufs, poi, out, tile_schedule):
    nbufs = len(abufs)
    row0 = 0
    for i, r in enumerate(tile_schedule):
        a = abufs[i % nbufs]
        b = bbufs[i % nbufs]
        rows = r * 128
        # load: HBM contiguous (24 KB/partition) -> SBUF contiguous
        src = poi[row0 : row0 + rows, :].rearrange("(p q) d -> p (q d)", q=r)
        nc.sync.dma_start(out=a[:, 0 : r * DPOI], in_=src)
        # leaky_relu(x) = max(0.2*x, x), interleaving into B's poi slots
        av = a[:, 0 : r * DPOI].rearrange("p (q d) -> p q d", q=r)
        bv = b[:].rearrange("p (q c) -> p q c", c=DOUT)
        nc.vector.scalar_tensor_tensor(
            out=bv[:, 0:r, 0:DPOI],
            in0=av,
            scalar=0.2,
            in1=av,
            op0=mybir.AluOpType.mult,
            op1=mybir.AluOpType.max,
        )
        # store: SBUF contiguous -> HBM contiguous (48 KB/partition).
        # Alternate the two HWDGE rings (sync/scalar): a single ring's FIFO
        # throttles the 64 MB store stream; split, the kernel runs at the
        # per-NC HBM write roofline (~180 us/pass vs ~285 single-ring).
        dst = out[row0 : row0 + rows, :].rearrange("(p q) c -> p (q c)", q=r)
        eng = nc.sync if i % 2 else nc.scalar
        eng.dma_start(out=dst, in_=b[:, 0 : r * DOUT])
        row0 += rows


def _build_program(pad_rows, tile_schedule, nbufs, repeats=1):
    import concourse.bacc as bacc
    import concourse.mybir as mybir
    from concourse.tile import TileContext

    f32 = mybir.dt.float32
    nc = bacc.Bacc()
    poi = nc.declare_dram_parameter("poi", [pad_rows, DPOI], f32, isOutput=False)
    ublk = nc.declare_dram_parameter("ublk", [128, DU], f32, isOutput=False)
    out = nc.declare_dram_parameter("out", [pad_rows, DOUT], f32, isOutput=True)

    rmax = max(tile_schedule)
    with TileContext(nc) as tc:
        with (
            tc.tile_pool(name="abuf", bufs=1) as apool,
            tc.tile_pool(name="bbuf", bufs=1) as bpool,
            tc.tile_pool(name="ubuf", bufs=1) as upool,
        ):
            usr = upool.tile([128, DU], f32)
            nc.sync.dma_start(out=usr[:], in_=ublk[:])

            abufs = [
                apool.tile([128, rmax * DPOI], f32, name=f"abuf{i}")
                for i in range(nbufs)
            ]
            bbufs = [
                bpool.tile([128, rmax * DOUT], f32, name=f"bbuf{i}")
                for i in range(nbufs)
            ]
            # Pre-fill the user-embedding slots of every B buffer once:
            # seed row-slot 0 from usr, then doubling copies, all on DVE so
            # each store's producers live on one engine.
            for b in bbufs:
                bv = b[:].rearrange("p (q c) -> p q c", c=DOUT)
                nc.vector.tensor_copy(
                    out=bv[:, 0:1, DPOI:DOUT],
                    in_=usr[:].rearrange("p (q c) -> p q c", q=1),
                )
                q = 1
                while q < rmax:
                    step = min(q, rmax - q)
                    nc.vector.tensor_copy(
                        out=bv[:, q : q + step, DPOI:DOUT],
                        in_=bv[:, 0:step, DPOI:DOUT],
                    )
                    q += step

            for _ in range(repeats):
                _emit_pass(nc, mybir, abufs, bbufs, poi, out, tile_schedule)
    nc.compile()
    return nc


def _get_program(pad_rows, tile_schedule, nbufs, repeats=1):
    key = (pad_rows, tuple(tile_schedule), nbufs, repeats)
    if key not in _prog_cache:
        _prog_cache[key] = _build_program(pad_rows, tile_schedule, nbufs, repeats)
    return _prog_cache[key]


def _prepare(hot, reg, user, rows_per_core, pad_rows, tile_schedule, nbufs, repeats=1):
    nc = _get_program(pad_rows, tile_schedule, nbufs, repeats)
    poi_full = np.concatenate(
        [np.ascontiguousarray(hot), np.ascontiguousarray(reg)], axis=1
    ).astype(np.float32, copy=False)
    ublk = np.broadcast_to(
        np.asarray(user, dtype=np.float32).reshape(1, DU), (128, DU)
    ).copy()
    in_maps = []
    for c in range(NCORES):
        sl = poi_full[c * rows_per_core : (c + 1) * rows_per_core]
        if pad_rows != rows_per_core:
            p = np.zeros((pad_rows, DPOI), np.float32)
            p[:rows_per_core] = sl
        else:
            p = np.ascontiguousarray(sl)
        in_maps.append({"poi": p, "ublk": ublk})
    return nc, in_maps


def _run(hot, reg, user, rows_per_core, pad_rows, tile_schedule, nbufs, **spmd_kwargs):
    from concourse.bass_utils import run_bass_kernel_spmd

    nc, in_maps = _prepare(
        hot, reg, user, rows_per_core, pad_rows, tile_schedule, nbufs
    )
    res = run_bass_kernel_spmd(nc, in_maps, list(range(NCORES)), **spmd_kwargs)
    outs = [res.results[c]["out"][:rows_per_core] for c in range(NCORES)]
    return np.concatenate(outs, axis=0), res


def kernel(hotness_embedding_list, region_embedding_list, user_embedding):
    out, _ = _run(
        hotness_embedding_list,
        region_embedding_list,
        user_embedding,
        ROWS_PER_CORE,
        PAD_ROWS,
        TILE_SCHEDULE,
        NBUFS,
    )
    return out
